# revision 6
# baseline (speedup 1.0000x reference)
"""Trainium2 Bass kernel v4 for patch-attention (nn_Attention_58755152609998).

Per core: 4 examples as 2 pairs. Stages per pair:
  load x (quad tiles) -> PE fp32 transpose -> xT fp8 -> fp8-DoubleRow QKV
  -> psum copies: q/k chunks -> fp8 buf (chunk (p,cc) at cc*3072+p*64),
     v chunks -> bf16 buf (chunk j=6*(p-40)+cc at j*128+b*64, pair-interleaved)
  -> per head: fp8-DR scores (both examples in one [128,64] psum), direct
     exp softmax (logits bounded) with fused sum, block-diag attn bf16,
     attnT + V^T via batched DMA transposes,
     O^T = v_tok-chunk^T @ attnT-bd into [128,1024] psums (one per c-half),
     scattered to a rolling 2-head oT window (raster-interleaved)
  -> per head-pair k: proj (stat=oT window, mov=w_proj bf16, bias folded
     into psum via ones-matmul), y bf16, one DMA per (example, k).
Output dram tensor is bf16 (host converts to fp32).

v4: front_end/attention are generators; attention(0) is emission-interleaved
with front_end(1) so the copy-bound QKV phase overlaps the PE-bound attention
phase. DMAs are spread across the SP and Pool queues; psum->sbuf copies are
spread across DVE/Act/Pool.
"""

import numpy as np

B_GLOBAL = 32
N_CORES = 8
B_LOC = B_GLOBAL // N_CORES
C = 256
H = 8
TOK = 4096
SCALE = float((32 * 64) ** -0.5)

import os

# engine picks: v=DVE a=Act(scalar) p=Pool(gpsimd)
ENGSEL = {
    "A": os.environ.get("SEL_A", "va" * 48),    # qk chunk copies (384 rows)
    "B": os.environ.get("SEL_B", "av" * 48),    # v chunk copies (192 rows)
    "C": os.environ.get("SEL_C", "va" * 32),    # xT copies (1024/384 rows)
    "F": os.environ.get("SEL_F", "av" * 32),    # O^T copies (1024 rows)
    "G": os.environ.get("SEL_G", "vaa" * 22),   # y copies (1024 rows)
}
# DMA queue picks: s=sync(SP) g=gpsimd(Pool)
DMASEL = {
    "X": os.environ.get("SEL_X", "sgsg"),       # x loads
    "Y": os.environ.get("SEL_Y", "s"),          # y stores
    "V": "s",              # v_tok transposes (HWDGE only)
    "T": "s",              # aT transposes (HWDGE only)
}


def _build_nc():
    import concourse.bass as bass
    import concourse.bacc as bacc
    import concourse.tile as tile
    from concourse import mybir
    from concourse.masks import make_identity

    fp32 = mybir.dt.float32
    bf16 = mybir.dt.bfloat16
    fp8 = mybir.dt.float8e4
    DR = mybir.MatmulPerfMode.DoubleRow

    nc = bacc.Bacc("TRN2", target_bir_lowering=False, debug=False,
                   enable_asserts=False, num_devices=N_CORES)

    x_t = nc.dram_tensor("x", [B_LOC, 64, 64, C], fp32, kind="ExternalInput")
    wq_t = nc.dram_tensor("w_qkv", [C, 3 * C], fp32, kind="ExternalInput")
    wp_t = nc.dram_tensor("w_proj", [C, C], fp32, kind="ExternalInput")
    bp_t = nc.dram_tensor("b_proj", [C], fp32, kind="ExternalInput")
    out_t = nc.dram_tensor("out", [B_LOC, 64, 64, C], bf16,
                           kind="ExternalOutput")

    EX = TOK * C

    def pick(site, idx):
        ch = ENGSEL[site][idx % len(ENGSEL[site])]
        return {"v": nc.vector, "a": nc.scalar, "p": nc.gpsimd}[ch]

    def dpick(site, idx):
        ch = DMASEL[site][idx % len(DMASEL[site])]
        return {"s": nc.sync, "g": nc.gpsimd}[ch]

    def eng_copy(eng, out, in_):
        if eng is nc.scalar:
            nc.scalar.copy(out=out, in_=in_)
        else:
            eng.tensor_copy(out=out, in_=in_)

    with tile.TileContext(nc) as tc:
        with (
            tc.tile_pool(name="consts", bufs=1) as consts,
            tc.tile_pool(name="xin", bufs=6) as xin_pool,
            tc.tile_pool(name="xT", bufs=2) as xT_pool,
            tc.tile_pool(name="qk", bufs=2) as qk_pool,
            tc.tile_pool(name="vbuf", bufs=2) as v_pool,
            tc.tile_pool(name="vtok", bufs=2) as vtok_pool,
            tc.tile_pool(name="attn", bufs=4) as attn_pool,
            tc.tile_pool(name="oTw", bufs=2) as oT_pool,
            tc.tile_pool(name="y", bufs=2) as y_pool,
            tc.tile_pool(name="ps_big", bufs=3, space="PSUM") as ps_big,
            tc.tile_pool(name="ps_sc", bufs=2, space="PSUM") as ps_sc_pool,
        ):
            ident_f = consts.tile([128, 128], fp32, name="ident_f",
                                  tag="ident_f")
            make_identity(nc, ident_f[:])
            ident_b = consts.tile([128, 128], bf16, name="ident_b",
                                  tag="ident_b")
            make_identity(nc, ident_b[:])

            w8 = consts.tile([128, 2 * 768], fp8, name="w8", tag="w8")
            for ch in range(2):
                tf = xin_pool.tile([128, 1024], fp32, name=f"wqf{ch}",
                                   tag="xq")
                nc.sync.dma_start(out=tf[:, 0:768],
                                  in_=wq_t.ap()[ch * 128:(ch + 1) * 128, :])
                nc.vector.tensor_copy(out=w8[:, ch * 768:(ch + 1) * 768],
                                      in_=tf[:, 0:768])
            wp = consts.tile([128, 2 * 256], bf16, name="wp", tag="wp")
            for ch in range(2):
                tf = xin_pool.tile([128, 1024], fp32, name=f"wpf{ch}",
                                   tag="xq")
                nc.sync.dma_start(out=tf[:, 0:256],
                                  in_=wp_t.ap()[ch * 128:(ch + 1) * 128, :])
                nc.vector.tensor_copy(out=wp[:, ch * 256:(ch + 1) * 256],
                                      in_=tf[:, 0:256])
            wqb = consts.tile([128, 2 * 768], bf16, name="wqb", tag="wqb")
            for ch in range(2):
                tf = xin_pool.tile([128, 1024], fp32, name=f"wqb{ch}",
                                   tag="xq")
                nc.sync.dma_start(out=tf[:, 0:768],
                                  in_=wq_t.ap()[ch * 128:(ch + 1) * 128, :])
                nc.vector.tensor_copy(out=wqb[:, ch * 768:(ch + 1) * 768],
                                      in_=tf[:, 0:768])
            ones_col = consts.tile([1, 128], bf16, name="ones_col",
                                   tag="ones_col")
            nc.gpsimd.memset(ones_col, 1.0)
            b_rowf = consts.tile([1, C], fp32, name="b_rowf", tag="b_rowf")
            nc.sync.dma_start(
                out=b_rowf,
                in_=bass.AP(tensor=bp_t, offset=0, ap=[[0, 1], [1, C]]))
            b_row = consts.tile([1, 2 * C], bf16, name="b_row", tag="b_row")
            nc.vector.tensor_copy(out=b_row[:, 0:C], in_=b_rowf)
            nc.vector.tensor_copy(out=b_row[:, C:2 * C], in_=b_rowf)

            def w8_stat(i):
                return bass.AP(tensor=w8.tensor, offset=w8.offset + i * 128,
                               ap=[w8.ap[0], [768, 2], [1, 128]])

            qk_all = {}
            v_all = {}

            def front_end(pair):
                qk_sb = qk_all.setdefault(pair, [])
                v_sb = v_pool.tile([128, 144 * 128], bf16, name=f"v_{pair}",
                                   tag="v")
                v_all[pair] = v_sb
                for b2 in range(2):
                    b = pair * 2 + b2
                    xT = xT_pool.tile([128, 2 * TOK], fp8, name=f"xT_{b}",
                                      tag="xT")
                    xTbf = xT_pool.tile([128, 2 * 8 * 192], bf16,
                                        name=f"xTbf_{b}", tag="xTbf")
                    for q in range(8):
                        xq = xin_pool.tile([128, 1024], fp32,
                                           name=f"xq_{b}_{q}", tag="xq")
                        with tc.high_priority():
                            dpick("X", b * 8 + q).dma_start(
                                out=xq,
                                in_=bass.AP(tensor=x_t,
                                            offset=b * EX + q * 512 * C,
                                            ap=[[C, 128], [128 * C, 4], [1, C]]))
                        ps_t = ps_big.tile([128, 1024], fp32,
                                           name=f"ps_xt_{b}_{q}", tag="big")
                        for ch in range(2):
                            for t4 in range(4):
                                nc.tensor.matmul(
                                    ps_t[:, ch * 512 + t4 * 128:
                                         ch * 512 + (t4 + 1) * 128],
                                    xq[:, t4 * 256 + ch * 128:
                                       t4 * 256 + ch * 128 + 128],
                                    ident_f, start=True, stop=True,
                                    is_transpose=True)
                        eng_copy(pick("C", b * 8 + q),
                                 out=bass.AP(
                                     tensor=xT.tensor,
                                     offset=xT.offset + q * 512,
                                     ap=[xT.ap[0], [TOK, 2], [1, 512]]),
                                 in_=ps_t)
                        eng_copy(pick("C", b * 8 + q + 3),
                                 out=bass.AP(
                                     tensor=xTbf.tensor,
                                     offset=xTbf.offset + q * 192,
                                     ap=[xTbf.ap[0], [8 * 192, 2], [1, 192]]),
                                 in_=bass.AP(
                                     tensor=ps_t.tensor,
                                     offset=ps_t.offset + 320,
                                     ap=[ps_t.ap[0], [512, 2], [1, 192]]))
                        if q % 2 == 1:
                            yield

                    qk = qk_pool.tile([128, 6 * 3072], fp8, name=f"qk_{b}",
                                      tag="qk")
                    qk_sb.append(qk)
                    for i in range(6):
                        for t2 in range(4):
                            ps_q = ps_big.tile([128, 1024], fp32,
                                               name=f"ps_q_{b}_{i}_{t2}",
                                               tag="big")
                            for s2 in range(2):
                                tt = t2 * 2 + s2
                                rhs = bass.AP(
                                    tensor=xT.tensor,
                                    offset=xT.offset + tt * 512,
                                    ap=[xT.ap[0], [TOK, 2], [1, 384]])
                                nc.tensor.matmul(
                                    ps_q[:, s2 * 512:s2 * 512 + 384],
                                    w8_stat(i), rhs, start=True, stop=True,
                                    perf_mode=DR)
                                for ch in range(2):
                                    nc.tensor.matmul(
                                        ps_q[:, s2 * 512 + 320:
                                             s2 * 512 + 512],
                                        wqb[:, ch * 768 + i * 128:
                                            ch * 768 + (i + 1) * 128],
                                        xTbf[:, ch * 1536 + tt * 192:
                                             ch * 1536 + (tt + 1) * 192],
                                        start=(ch == 0), stop=(ch == 1),
                                        skip_group_check=True)
                            for s2 in range(2):
                                tt = t2 * 2 + s2
                                eng_copy(
                                    pick("A", b * 24 + i * 8 + t2 * 2 + s2),
                                    out=bass.AP(
                                        tensor=qk.tensor,
                                        offset=(qk.offset + i * 3072
                                                + tt * 8),
                                        ap=[qk.ap[0], [512, 6], [1, 8],
                                            [64, 8]]),
                                    in_=bass.AP(
                                        tensor=ps_q.tensor,
                                        offset=ps_q.offset + s2 * 512,
                                        ap=[ps_q.ap[0], [1, 384]]))
                                eng_copy(
                                    pick("B", b * 24 + i * 8 + t2 * 2 + s2),
                                    out=bass.AP(
                                        tensor=v_sb.tensor,
                                        offset=(v_sb.offset + i * 128
                                                + b2 * 64 + tt * 8),
                                        ap=[v_sb.ap[0], [6144, 3], [1, 8],
                                            [768, 8]]),
                                    in_=bass.AP(
                                        tensor=ps_q.tensor,
                                        offset=ps_q.offset + s2 * 512 + 320,
                                        ap=[ps_q.ap[0], [1, 192]]))
                        yield

            def attention(pair):
                qk_sb = qk_all[pair]
                v_sb = v_all[pair]
                # all scores up-front: releases qk buffers early
                sc_tiles = [ps_sc_pool.tile([64, 512], fp32,
                                            name=f"sc_{pair}_{g}", tag="sc")
                            for g in range(2)]
                for h in range(H):
                    sc_tile = sc_tiles[h // 4]
                    for b2 in range(2):
                        qk = qk_sb[b2]
                        ps_sc = sc_tile[:, (h % 4) * 128 + b2 * 64:
                                        (h % 4) * 128 + b2 * 64 + 64]
                        for j in range(8):
                            mq = 16 * h + 2 * j
                            mk = 128 + 16 * h + 2 * j
                            lhs = bass.AP(
                                tensor=qk.tensor,
                                offset=(qk.offset + (mq % 6) * 3072
                                        + (mq // 6) * 64),
                                ap=[qk.ap[0], [3072, 2], [1, 64]])
                            rhs = bass.AP(
                                tensor=qk.tensor,
                                offset=(qk.offset + (mk % 6) * 3072
                                        + (mk // 6) * 64),
                                ap=[qk.ap[0], [3072, 2], [1, 64]])
                            nc.tensor.matmul(
                                ps_sc, lhs, rhs,
                                start=(j == 0), stop=(j == 7),
                                perf_mode=DR, skip_group_check=True)
                    if h % 2 == 1:
                        yield

                for k in range(4):          # head pairs
                    oTw = [oT_pool.tile([128, 2048], bf16,
                                        name=f"oTw{half}_{pair}_{k}",
                                        tag=f"oTw{half}")
                           for half in range(2)]
                    for h2 in range(2):
                        h = 2 * k + h2
                        sc_tile = sc_tiles[h // 4]
                        pe = attn_pool.tile([64, 128], fp32, name="pe",
                                            tag="pe")
                        s_sum = attn_pool.tile([64, 2], fp32, name="s_sum",
                                               tag="s_sum")
                        nc.scalar.activation(
                            pe,
                            sc_tile[:, (h % 4) * 128:(h % 4) * 128 + 128],
                            mybir.ActivationFunctionType.Exp,
                            scale=SCALE)
                        for b2 in range(2):
                            nc.vector.reduce_sum(
                                s_sum[:, b2:b2 + 1],
                                pe[:, b2 * 64:(b2 + 1) * 64],
                                axis=mybir.AxisListType.X)
                        r_sum = attn_pool.tile([64, 2], fp32, name="r_sum",
                                               tag="r_sum")
                        nc.vector.reciprocal(r_sum, s_sum)
                        acat = attn_pool.tile([64, 128], bf16, name="acat",
                                              tag="acat")
                        for b2 in range(2):
                            nc.gpsimd.tensor_scalar_mul(
                                acat[:, b2 * 64:(b2 + 1) * 64],
                                pe[:, b2 * 64:(b2 + 1) * 64],
                                r_sum[:, b2:b2 + 1])
                        aT = attn_pool.tile([128, 64], bf16, name="aT",
                                            tag="aT")
                        dpick("T", pair * 8 + h).dma_start_transpose(
                            out=aT, in_=acat)
                        attnT = attn_pool.tile([128, 128], bf16, name="attnT",
                                               tag="attnT")
                        nc.gpsimd.memset(attnT[0:64, 64:128], 0.0)
                        nc.gpsimd.memset(attnT[64:128, 0:64], 0.0)
                        nc.gpsimd.tensor_copy(out=attnT[0:64, 0:64],
                                              in_=aT[0:64, :])
                        nc.gpsimd.tensor_copy(out=attnT[64:128, 64:128],
                                              in_=aT[64:128, :])

                        v_tok = vtok_pool.tile([128, 2048], bf16,
                                               name=f"vtok_{pair}_{h}",
                                               tag="vtok")
                        j0 = 16 * h + 16
                        dpick("V", pair * 8 + h).dma_start_transpose(
                            out=bass.AP(tensor=v_tok.tensor,
                                        offset=v_tok.offset,
                                        ap=[v_tok.ap[0], [128, 16], [1, 128]]),
                            in_=v_sb[:, j0 * 128:(j0 + 16) * 128])

                        for half in range(2):
                            ps_oc = ps_big.tile([128, 1024], fp32,
                                                name=f"ps_o_{h}_{half}",
                                                tag="big")
                            for p2l in range(8):
                                jp = 2 * p2l + half
                                nc.tensor.matmul(
                                    ps_oc[:, p2l * 128:(p2l + 1) * 128],
                                    v_tok[:, jp * 128:(jp + 1) * 128],
                                    attnT, start=True, stop=True)
                            dst = oTw[half]
                            eng_copy(
                                pick("F", (pair * 8 + h) * 2 + half),
                                out=bass.AP(
                                    tensor=dst.tensor,
                                    offset=dst.offset + h2 * 64,
                                    ap=[dst.ap[0], [1, 8], [128, 16],
                                        [8, 8]]),
                                in_=bass.AP(
                                    tensor=ps_oc.tensor,
                                    offset=ps_oc.offset,
                                    ap=[ps_oc.ap[0], [128, 8], [8, 16],
                                        [1, 8]]))
                        yield

                    # proj + store for this head pair (rows 8gi+2k, 8gi+2k+1)
                    for b2 in range(2):
                        b = pair * 2 + b2
                        yq = y_pool.tile([128, 2048], bf16,
                                         name=f"y_{b}_{k}", tag="y")
                        for gq in range(2):     # gi quads
                            ps_y = ps_big.tile([128, 1024], fp32,
                                               name=f"ps_y_{b}_{k}_{gq}",
                                               tag="big")
                            for s2 in range(2):
                                nc.tensor.matmul(
                                    ps_y[:, s2 * 512:(s2 + 1) * 512],
                                    ones_col, b_row, start=True, stop=False,
                                    skip_group_check=True)
                            for gl in range(4):
                                gi = gq * 4 + gl
                                for half in range(2):
                                    nc.tensor.matmul(
                                        ps_y[:, gl * 256:(gl + 1) * 256],
                                        oTw[half][:, b2 * 1024 + gi * 128:
                                                  b2 * 1024 + (gi + 1) * 128],
                                        wp[:, half * 256:(half + 1) * 256],
                                        start=False,
                                        stop=(half == 1),
                                        skip_group_check=True)
                            eng_copy(pick("G", (b * 4 + k) * 2 + gq),
                                     out=yq[:, gq * 1024:(gq + 1) * 1024],
                                     in_=ps_y)
                        dpick("Y", b * 4 + k).dma_start(
                            out=bass.AP(
                                tensor=out_t,
                                offset=b * EX + 2 * k * 64 * C,
                                ap=[[C, 128], [8 * 64 * C, 8], [1, C]]),
                            in_=yq)
                        yield

            def drain(gen):
                for _ in gen:
                    pass

            def interleave(g1, g2, r1=1, r2=1):
                """Alternate emission: r1 chunks of g1, then r2 of g2."""
                done1 = done2 = False
                while not (done1 and done2):
                    for _ in range(r1):
                        if not done1:
                            try:
                                next(g1)
                            except StopIteration:
                                done1 = True
                    for _ in range(r2):
                        if not done2:
                            try:
                                next(g2)
                            except StopIteration:
                                done2 = True

            _order = os.environ.get("ORDER", "interleave")
            if _order == "seq":
                drain(front_end(0))
                drain(front_end(1))
                drain(attention(0))
                drain(attention(1))
            else:
                drain(front_end(0))
                interleave(front_end(1), attention(0), 2, 1)
                drain(attention(1))

    nc.compile()
    return nc


_NC_CACHE = None


def kernel(x, w_qkv, w_proj, b_proj):
    global _NC_CACHE
    from concourse import bass_utils

    x = np.ascontiguousarray(np.asarray(x, dtype=np.float32))
    w_qkv = np.ascontiguousarray(np.asarray(w_qkv, dtype=np.float32))
    w_proj = np.ascontiguousarray(np.asarray(w_proj, dtype=np.float32))
    b_proj = np.ascontiguousarray(np.asarray(b_proj, dtype=np.float32))

    if _NC_CACHE is None:
        _NC_CACHE = _build_nc()
    nc = _NC_CACHE

    in_maps = []
    for c in range(N_CORES):
        in_maps.append({
            "x": x[c * B_LOC:(c + 1) * B_LOC],
            "w_qkv": w_qkv,
            "w_proj": w_proj,
            "b_proj": b_proj,
        })
    res = bass_utils.run_bass_kernel_spmd(nc, in_maps, list(range(N_CORES)))
    out = np.concatenate([np.asarray(r["out"], dtype=np.float32)
                          for r in res.results], axis=0)
    return out


if __name__ == "__main__":
    nc = _build_nc()
    print("built ok")
